# revision 20
# baseline (speedup 1.0000x reference)
"""Soft-DTW loss kernel for Trainium2 (Bass/Tile), 8-core data-parallel.

Problem: B=64 samples; per sample cost C = cdist(pred_b, target_b) (512x512),
then soft-DTW DP (gamma=1) over C; loss = mean_b(dtw_b / 1024).

Strategy
--------
Data-parallel: 8 samples per core. Per core the DP runs in the exp domain:
  E[i,j] = EC[i,j] * (E[i-1,j] + E[i-1,j-1] + E[i,j-1]),  EC = exp(-C)
which makes the serial recurrence pure multiply-add. Rows are processed with
`tensor_tensor_scan` (state = (v + state) * EC along the free dim); the 512
columns are split into 15 chunks of width 35 assigned to SBUF partitions
(partition = 16*b + s, s=0 ghost/boundary, s=1..15 chunk). A (row x chunk)
wavefront runs 526 steps; cross-chunk carries move one partition-slot via
stream_shuffle (within 16-slot groups, so quadrant-local). Dynamic range is
handled by multiplying the state by exp(4*kappa_b) every 4 steps where
kappa_b = r_hat_b/524 and r_hat_b = A*trace(C_b) + B_ is a per-sample estimate
of the final DTW value (fit offline; residual +-25 nats over the batch
distribution), plus a clamp that only truncates provably negligible paths.
Final: loss_b = (r_hat_b - ln z_b) / 1024, reduced to the scalar mean on host
(the gather step).
"""

import numpy as np
from contextlib import ExitStack

import concourse.bass as bass
import concourse.tile as tile
from concourse import bacc, mybir
from concourse.bass_utils import run_bass_kernel_spmd

f32 = mybir.dt.float32
AL = mybir.AluOpType
AF = mybir.ActivationFunctionType

B, S, F = 64, 512, 128
NCORES = 8
BL = B // NCORES          # 8 samples per core
W = 35                    # chunk width (cols per partition-slot)
NS = 15                   # chunks per sample; 15*35=525 >= 512
NSTEP = S + NS - 1        # 526 wavefront steps
SLOT = W + 2              # ring slot layout: [halo | 35 data | init]
ECLEN = (NSTEP + 1) * W   # skewed EC buffer length per partition
BIG = 1.0e30              # pad cost -> EC = exp(-BIG) = 0
CLAMP = 3.0e32            # state clamp (truncates negligible paths only)
# r_hat = TR_A * trace(C) + TR_B  (offline fit, resid +-25 nats over 64 samples)
TR_A = 0.7264
TR_B = 2153.3  # 2203.3 - 50 band-centering offset (Phi target = rhat - 50)
NAPPLY = NSTEP // 4       # number of scale steps (t = 4, 8, ..., 524) = 131
# kappa chosen so total applied log-scale == r_hat: 4*kappa*NAPPLY = r_hat


def build_core_program(debug_outputs=False):
    nc = bacc.Bacc("TRN2", target_bir_lowering=False, debug=False,
                   num_devices=NCORES)
    pred_d = nc.dram_tensor("pred", [BL, S, F], f32, kind="ExternalInput")
    targ_d = nc.dram_tensor("target", [BL, S, F], f32, kind="ExternalInput")
    # outputs: final exp-domain value z_b and the applied log-scale rhat_b;
    # host computes loss_b = (rhat_b - ln z_b)/1024 (ACT Ln is inaccurate for
    # tiny arguments, so the ln runs on host during the gather).
    zf_d = nc.dram_tensor("zf", [1, BL], f32, kind="ExternalOutput")
    rhat_d = nc.dram_tensor("rhat", [1, BL], f32, kind="ExternalOutput")

    RT = S // 128  # 4 row tiles per sample

    with tile.TileContext(nc) as tc, ExitStack() as ctx:
        pool = ctx.enter_context(tc.tile_pool(name="persist", bufs=1))
        spool = ctx.enter_context(tc.tile_pool(name="stage", bufs=2))
        ppool = ctx.enter_context(tc.tile_pool(name="psum", bufs=2, space="PSUM"))
        ppool_t = ctx.enter_context(tc.tile_pool(name="psum_t", bufs=2, space="PSUM"))
        ppool_s = ctx.enter_context(tc.tile_pool(name="psum_small", bufs=1, space="PSUM"))

        # ---------------- persistent tiles ----------------
        ec = pool.tile([128, ECLEN], f32, tag="ec")          # skewed cost -> EC
        zr = pool.tile([128, 3, SLOT], f32, tag="zr")        # state ring
        vt = pool.tile([128, W], f32, tag="vt")              # pair-sum v
        g4 = pool.tile([128, 1], f32, tag="g4")              # per-partition exp(4k)
        ident = pool.tile([128, 128], f32, tag="ident")      # identity for PE T
        selm = pool.tile([128, BL], f32, tag="selm")         # final gather matrix
        bmask = pool.tile([128, BL], f32, tag="bmask")       # [p,b] = (p>>4)==b
        qdiag = pool.tile([128, RT * BL], f32, tag="qdiag")  # diag(C) pieces
        trc = pool.tile([1, BL], f32, tag="trc")             # trace per sample
        g4f = pool.tile([1, BL], f32, tag="g4f")             # exp(rhat/131) [1,B]
        g4bc = pool.tile([128, BL], f32, tag="g4bc")
        zfin = pool.tile([1, BL], f32, tag="zfin")
        rhat_t = pool.tile([1, BL], f32, tag="rhat")

        # ---------------- constants ----------------
        from concourse import masks
        masks.make_identity(nc, ident[:])
        # selm[k, m] = 1 where k == 16*m + 15
        nc.gpsimd.memset(selm[:], 0.0)
        nc.gpsimd.affine_select(
            out=selm[:], in_=selm[:], compare_op=AL.not_equal, fill=1.0,
            base=-15, pattern=[[-16, BL]], channel_multiplier=1)
        # bmask[p, b] = 1 where 0 <= p - 16*b <= 15
        nc.gpsimd.memset(bmask[:], 1.0)
        nc.gpsimd.affine_select(
            out=bmask[:], in_=bmask[:], compare_op=AL.is_ge, fill=0.0,
            base=0, pattern=[[-16, BL]], channel_multiplier=1)
        nc.gpsimd.affine_select(
            out=bmask[:], in_=bmask[:], compare_op=AL.is_ge, fill=0.0,
            base=15, pattern=[[16, BL]], channel_multiplier=-1)

        # pad/ghost fill: EC buffer starts as BIG cost everywhere
        nc.gpsimd.memset(ec[:], BIG)
        nc.gpsimd.memset(zr[:], 0.0)
        nc.gpsimd.memset(vt[:], 0.0)
        ones = pool.tile([128, 1], f32, tag="ones")
        nc.gpsimd.memset(ones[:], 1.0)

        # DP corner seed: shuffle at t=1 reads slot_prev(=slot 0 of ring idx 2)
        # ... we define slot index for step t as t % 3; at t=1 prev slot is 0,
        # p2 slot is 2. The t=1 halo reads Z_{t-1}=slot0 col W via... see loop:
        # halo source is Z_{p2}[:, W] = slot 2; init source Z_{prev}=slot 0.
        # Corner: E[0, col0] = 1 must arrive as halo of chunk 1 at t=1, i.e.
        # ghost slot (s=0) of slot-ring "t-1 state" at data col W.
        # At t=1: halo <- shuffle from Z_{(t-2)%3 = 2}[:, W]. So seed slot 2.
        selm0 = pool.tile([128, BL], f32, tag="selm0")
        nc.gpsimd.memset(selm0[:], 0.0)
        nc.gpsimd.affine_select(
            out=selm0[:], in_=selm0[:], compare_op=AL.not_equal, fill=1.0,
            base=0, pattern=[[-16, BL]], channel_multiplier=1)
        nc.vector.tensor_reduce(zr[:, 2, W : W + 1], selm0[:],
                                axis=mybir.AxisListType.X, op=AL.add)

        # ================= bulk phase: per-sample cost -> EC =================
        for b in range(BL):
            pn = spool.tile([128, RT, F], f32, tag="pn")
            tn = spool.tile([128, RT, F], f32, tag="tn")
            nc.sync.dma_start(pn[:], pred_d[b].rearrange("(a p) f -> p a f", p=128))
            nc.sync.dma_start(tn[:], targ_d[b].rearrange("(a p) f -> p a f", p=128))

            ptr = spool.tile([128, RT, 128], f32, tag="ptr")   # -2 * pred^T
            ttr = spool.tile([128, RT, 128], f32, tag="ttr")   # target^T
            for rt in range(RT):
                ps = ppool_t.tile([128, 128], f32, tag="pst")
                nc.tensor.matmul(ps[:], pn[:, rt], ident[:],
                                 start=True, stop=True, is_transpose=True)
                nc.scalar.activation(ptr[:, rt], ps[:], AF.Copy, scale=-2.0)
                ps2 = ppool_t.tile([128, 128], f32, tag="pst")
                nc.tensor.matmul(ps2[:], tn[:, rt], ident[:],
                                 start=True, stop=True, is_transpose=True)
                nc.scalar.copy(ttr[:, rt], ps2[:])

            # x2[p, rt] = sum_f pred^2 (per DP row); dump tile reused
            x2 = spool.tile([128, RT], f32, tag="x2")
            dump = spool.tile([128, F], f32, tag="dump")
            for rt in range(RT):
                nc.vector.scalar_tensor_tensor(
                    dump[:], pn[:, rt], 1.0, pn[:, rt],
                    op0=AL.mult, op1=AL.mult, accum_out=x2[:, rt : rt + 1])
            # y2 flat [1, 512] via ones-matmul over (target^T)^2
            tsq = spool.tile([128, RT * 128], f32, tag="tsq")
            ttr_flat = ttr[:].rearrange("p a f -> p (a f)")
            nc.vector.tensor_mul(tsq[:], ttr_flat, ttr_flat)
            y2p = ppool_s.tile([1, S], f32, tag="y2p")
            nc.tensor.matmul(y2p[:], ones[:], tsq[:], start=True, stop=True)
            y2s = spool.tile([1, S], f32, tag="y2s")
            nc.scalar.copy(y2s[:], y2p[:])
            y2b = spool.tile([128, S], f32, tag="y2b")
            nc.gpsimd.partition_broadcast(y2b[:], y2s[:])

            d2s = spool.tile([128, RT, S], f32, tag="d2s")
            for rt in range(RT):
                mm = ppool.tile([128, S], f32, tag="mm")
                nc.tensor.matmul(mm[:], ptr[:, rt], ttr_flat,
                                 start=True, stop=True)
                # d2 = (-2xy + x2) + y2
                nc.vector.scalar_tensor_tensor(
                    d2s[:, rt], mm[:], x2[:, rt : rt + 1], y2b[:],
                    op0=AL.add, op1=AL.add)
                # diag piece: qdiag[p, rt*BL + b] = d2[p, rt*128 + p]
                nc.vector.scalar_tensor_tensor(
                    dump[:, 0:128], d2s[:, rt, rt * 128 : (rt + 1) * 128], 1.0,
                    ident[:], op0=AL.mult, op1=AL.mult,
                    accum_out=qdiag[:, rt * BL + b : rt * BL + b + 1])

            # scatter d2 -> skewed ec buffer (cost for now):
            # dest partition 16b+s, elem offset (rt*128 + p + s)*W + j
            for s in range(1, NS + 1):
                c0 = (s - 1) * W
                nj = min(W, S - c0)
                for rt in range(RT):
                    src = d2s[:, rt, c0 : c0 + nj]
                    base = ec[16 * b + s : 16 * b + s + 1,
                              (rt * 128 + s) * W : (rt * 128 + s) * W + 1]
                    dst = bass.AP(
                        base.tensor, base.offset,
                        [[base.ap[0][0], 1], [W, 128], [1, nj]])
                    eng = nc.sync if (s + rt) % 2 == 0 else nc.gpsimd
                    eng.dma_start(dst, src)

        # sqrt then exp over the whole skewed buffer (in place):
        # C = sqrt(d2); EC = exp(-C). Pad BIG -> sqrt=1e15 -> exp -> 0.
        nc.scalar.activation(ec[:], ec[:], AF.Sqrt)
        # trace: qdiag currently holds diag d2 -> sqrt, then ones-matmul
        nc.scalar.activation(qdiag[:], qdiag[:], AF.Sqrt)
        trp = ppool_s.tile([1, RT * BL], f32, tag="trp")
        nc.tensor.matmul(trp[:], ones[:], qdiag[:], start=True, stop=True)
        trs = pool.tile([1, RT * BL], f32, tag="trs")
        nc.scalar.copy(trs[:], trp[:])
        # sum the RT pieces per sample: cols rt*BL + b -> [1, BL, RT] reduce X
        nc.vector.tensor_reduce(
            trc[:], trs[:].rearrange("o (a b) -> o b a", a=RT),
            axis=mybir.AxisListType.X, op=AL.add)
        nc.scalar.activation(ec[:], ec[:], AF.Exp, scale=-1.0)

        # per-scale factor = exp(4*kappa) = exp(rhat/NAPPLY)
        nc.vector.tensor_scalar(g4f[:], trc[:], TR_A / NAPPLY, TR_B / NAPPLY,
                                op0=AL.mult, op1=AL.add)
        nc.scalar.activation(g4f[:], g4f[:], AF.Exp)
        nc.gpsimd.partition_broadcast(g4bc[:], g4f[:])
        gsel = spool.tile([128, BL], f32, tag="gsel")
        nc.vector.tensor_mul(gsel[:], g4bc[:], bmask[:])
        nc.vector.tensor_reduce(g4[:], gsel[:], axis=mybir.AxisListType.X,
                                op=AL.add)
        nc.vector.tensor_scalar(rhat_t[:], trc[:], TR_A, TR_B,
                                op0=AL.mult, op1=AL.add)

        # ================= serial wavefront =================
        shuf_mask = [(i if i % 16 == 0 else i - 1) for i in range(32)]
        for t in range(1, NSTEP + 1):
            cur, prev, p2 = t % 3, (t - 1) % 3, (t - 2) % 3
            # carries: halo (from Z_{t-2}) -> prev slot col 0;
            #          init (from Z_{t-1}) -> prev slot col W+1
            if p2 < prev:
                src = zr[:, p2 : prev + 1 : (prev - p2), W]
            else:
                src = zr[:, p2 :: (prev - p2), W]  # negative step
            nc.vector.stream_shuffle(zr[:, prev, 0 : SLOT : SLOT - 1], src,
                                     shuf_mask)
            if t % 4 == 1 and t > 1:
                # halo missed the scale applied at t-1
                nc.vector.tensor_scalar(zr[:, prev, 0:1], zr[:, prev, 0:1],
                                        g4[:], CLAMP, op0=AL.mult, op1=AL.min)
            # v = Z_prev[j] + Z_prev[j-1]  (halo sits at col 0)
            nc.vector.tensor_add(vt[:], zr[:, prev, 1 : W + 1], zr[:, prev, 0:W])
            if t % 4 == 0:
                nc.vector.tensor_scalar(vt[:], vt[:], g4[:], CLAMP,
                                        op0=AL.mult, op1=AL.min)
                nc.vector.tensor_scalar(zr[:, prev, W + 1 : W + 2],
                                        zr[:, prev, W + 1 : W + 2],
                                        g4[:], CLAMP, op0=AL.mult, op1=AL.min)
            nc.vector.tensor_tensor_scan(
                zr[:, cur, 1 : W + 1], vt[:], ec[:, t * W : t * W + W],
                zr[:, prev, W + 1 : W + 2], op0=AL.add, op1=AL.mult)

        # ================= finalize =================
        # answer: z at partition 16b+15, data col (511 - 14*35) = 21 -> slot col 22
        fcol = S - 1 - (NS - 1) * W  # 21
        fs = NSTEP % 3
        zp = ppool_s.tile([1, BL], f32, tag="zp")
        nc.tensor.matmul(zp[:], zr[:, fs, 1 + fcol : 2 + fcol], selm[:],
                         start=True, stop=True)
        nc.vector.tensor_copy(zfin[:], zp[:])
        nc.sync.dma_start(zf_d[:, :], zfin[:])
        nc.sync.dma_start(rhat_d[:, :], rhat_t[:])

    nc.compile()
    return nc


_NC_CACHE = {}


def _get_nc(debug_outputs=False):
    key = bool(debug_outputs)
    if key not in _NC_CACHE:
        _NC_CACHE[key] = build_core_program(debug_outputs=key)
    return _NC_CACHE[key]


def kernel(pred, target, _debug=False):
    pred = np.asarray(pred, dtype=np.float32)
    target = np.asarray(target, dtype=np.float32)
    nc = _get_nc(_debug)
    in_maps = []
    for c in range(NCORES):
        sl = slice(c * BL, (c + 1) * BL)
        in_maps.append({"pred": np.ascontiguousarray(pred[sl]),
                        "target": np.ascontiguousarray(target[sl])})
    res = run_bass_kernel_spmd(nc, in_maps, list(range(NCORES)))
    zf = np.concatenate([res.results[c]["zf"][0] for c in range(NCORES)])
    rhat = np.concatenate([res.results[c]["rhat"][0] for c in range(NCORES)])
    losses = (rhat.astype(np.float64) - np.log(zf.astype(np.float64))) / 1024.0
    if _debug:
        return np.float32(losses.mean()), {"z": zf, "rhat": rhat, "losses": losses}
    return np.float32(losses.mean())


if __name__ == "__main__":
    rng = np.random.default_rng(0)
    p = rng.standard_normal((B, S, F)).astype(np.float32)
    t = rng.standard_normal((B, S, F)).astype(np.float32)
    out, dbg = kernel(p, t, _debug=True)
    print("loss:", out)
    print("z:", dbg["z"][:8])
    print("rhat:", dbg["rhat"][:8])
    print("losses:", dbg["losses"][:8])


# revision 29
# speedup vs baseline: 236.4587x; 236.4587x over previous
"""Soft-DTW loss kernel for Trainium2 (Bass/Tile), 8-core data-parallel.

Problem: B=64 samples; per sample cost C = cdist(pred_b, target_b) (512x512),
then soft-DTW DP (gamma=1) over C; loss = mean_b(dtw_b / 1024).

Strategy
--------
Data-parallel: 8 samples per core. Per core the DP runs in the exp domain:
  E[i,j] = EC[i,j] * (E[i-1,j] + E[i-1,j-1] + E[i,j-1]),  EC = exp(-C)
which makes the serial recurrence pure multiply-add. Rows are processed with
`tensor_tensor_scan` (state = (v + state) * EC along the free dim); the 512
columns are split into 15 chunks of width 35 assigned to SBUF partitions
(partition = 16*b + s, s=0 ghost/boundary, s=1..15 chunk). A (row x chunk)
wavefront runs 526 steps; cross-chunk carries move one partition-slot via
stream_shuffle (within 16-slot groups, so quadrant-local). Dynamic range is
handled by multiplying the state by exp(4*kappa_b) every 4 steps where
kappa_b = r_hat_b/524 and r_hat_b = A*trace(C_b) + B_ is a per-sample estimate
of the final DTW value (fit offline; residual +-25 nats over the batch
distribution), plus a clamp that only truncates provably negligible paths.
Final: loss_b = (r_hat_b - ln z_b) / 1024, reduced to the scalar mean on host
(the gather step).
"""

import numpy as np
from contextlib import ExitStack

import concourse.bass as bass
import concourse.tile as tile
from concourse import bacc, mybir
from concourse.bass_utils import run_bass_kernel_spmd

f32 = mybir.dt.float32
AL = mybir.AluOpType
AF = mybir.ActivationFunctionType

B, S, F = 64, 512, 128
NCORES = 8
BL = B // NCORES          # 8 samples per core
W = 35                    # chunk width (cols per partition-slot)
NS = 15                   # chunks per sample; 15*35=525 >= 512
NSTEP = S // 2 + NS - 1   # 270 two-row wavefront steps
SLOT = 2 * (W + 1)        # ring slot: [c0|d0(35)|c1|d1(35)] (c=left carry)
JP = S + 2 * NS + 1       # j-major pitch: t' = r + 2s in [3, 542]
ECLEN = W * JP            # skewed EC buffer length per partition
BIG = 1.0e30              # pad cost -> EC = exp(-BIG) = 0
CLAMP = 3.0e32            # state clamp (truncates negligible paths only)
# r_hat = TR_A * trace(C) + TR_B  (offline fit, resid +-25 nats over 64 samples)
TR_A = 0.7264
TR_B = 2168.3  # 2203.3 - 35 band-centering offset (Phi target = rhat - 35)
NAPPLY = NSTEP // 2       # scale steps (even k in [2,270]) = 135
# kappa chosen so total applied log-scale == r_hat: 4*kappa*NAPPLY = r_hat


def build_core_program(debug_outputs=False):
    nc = bacc.Bacc("TRN2", target_bir_lowering=False, debug=False,
                   num_devices=NCORES)
    pred_d = nc.dram_tensor("pred", [BL, S, F], f32, kind="ExternalInput")
    targ_d = nc.dram_tensor("target", [BL, S, F], f32, kind="ExternalInput")
    # outputs: final exp-domain value z_b and the applied log-scale rhat_b;
    # host computes loss_b = (rhat_b - ln z_b)/1024 (ACT Ln is inaccurate for
    # tiny arguments, so the ln runs on host during the gather).
    zf_d = nc.dram_tensor("zf", [1, BL], f32, kind="ExternalOutput")
    rhat_d = nc.dram_tensor("rhat", [1, BL], f32, kind="ExternalOutput")

    RT = S // 128  # 4 row tiles per sample

    with tile.TileContext(nc) as tc, ExitStack() as ctx:
        pool = ctx.enter_context(tc.tile_pool(name="persist", bufs=1))
        spool = ctx.enter_context(tc.tile_pool(name="stage", bufs=2))
        ppool = ctx.enter_context(tc.tile_pool(name="psum", bufs=2, space="PSUM"))
        ppool_t = ctx.enter_context(tc.tile_pool(name="psum_t", bufs=2, space="PSUM"))
        ppool_s = ctx.enter_context(tc.tile_pool(name="psum_small", bufs=1, space="PSUM"))

        # ---------------- persistent tiles ----------------
        ec = pool.tile([128, ECLEN], f32, tag="ec")          # skewed cost -> EC
        zr = pool.tile([128, 3, SLOT], f32, tag="zr")        # state ring
        vt = pool.tile([128, W], f32, tag="vt")              # pair-sum v
        g4 = pool.tile([128, 1], f32, tag="g4")              # per-partition exp(4k)
        ident = pool.tile([128, 128], f32, tag="ident")      # identity for PE T
        selm = pool.tile([128, BL], f32, tag="selm")         # final gather matrix
        bmask = pool.tile([128, BL], f32, tag="bmask")       # [p,b] = (p>>4)==b
        qdiag = pool.tile([128, RT * BL], f32, tag="qdiag")  # diag(C) pieces
        trc = pool.tile([1, BL], f32, tag="trc")             # trace per sample
        g4f = pool.tile([1, BL], f32, tag="g4f")             # exp(rhat/131) [1,B]
        g4bc = pool.tile([128, BL], f32, tag="g4bc")
        zfin = pool.tile([1, BL], f32, tag="zfin")
        rhat_t = pool.tile([1, BL], f32, tag="rhat")

        # ---------------- constants ----------------
        from concourse import masks
        masks.make_identity(nc, ident[:])
        # selm[k, m] = 1 where k == 16*m + 15
        nc.gpsimd.memset(selm[:], 0.0)
        nc.gpsimd.affine_select(
            out=selm[:], in_=selm[:], compare_op=AL.not_equal, fill=1.0,
            base=-15, pattern=[[-16, BL]], channel_multiplier=1)
        # bmask[p, b] = 1 where 0 <= p - 16*b <= 15
        nc.gpsimd.memset(bmask[:], 1.0)
        nc.gpsimd.affine_select(
            out=bmask[:], in_=bmask[:], compare_op=AL.is_ge, fill=0.0,
            base=0, pattern=[[-16, BL]], channel_multiplier=1)
        nc.gpsimd.affine_select(
            out=bmask[:], in_=bmask[:], compare_op=AL.is_ge, fill=0.0,
            base=15, pattern=[[16, BL]], channel_multiplier=-1)

        # pad/ghost fill: EC buffer starts as BIG cost everywhere
        nc.gpsimd.memset(ec[:], BIG)
        nc.gpsimd.memset(zr[:], 0.0)
        nc.gpsimd.memset(vt[:], 0.0)
        ones = pool.tile([128, 1], f32, tag="ones")
        nc.gpsimd.memset(ones[:], 1.0)

        # DP corner seed: shuffle at t=1 reads slot_prev(=slot 0 of ring idx 2)
        # ... we define slot index for step t as t % 3; at t=1 prev slot is 0,
        # p2 slot is 2. The t=1 halo reads Z_{t-1}=slot0 col W via... see loop:
        # halo source is Z_{p2}[:, W] = slot 2; init source Z_{prev}=slot 0.
        # Corner: E[0, col0] = 1 must arrive as halo of chunk 1 at t=1, i.e.
        # ghost slot (s=0) of slot-ring "t-1 state" at data col W.
        # At t=1: halo <- shuffle from Z_{(t-2)%3 = 2}[:, W]. So seed slot 2.
        selm0 = pool.tile([128, BL], f32, tag="selm0")
        nc.gpsimd.memset(selm0[:], 0.0)
        nc.gpsimd.affine_select(
            out=selm0[:], in_=selm0[:], compare_op=AL.not_equal, fill=1.0,
            base=-1, pattern=[[-16, BL]], channel_multiplier=1)
        nc.vector.tensor_reduce(zr[:, 0, W + 1 : W + 2], selm0[:],
                                axis=mybir.AxisListType.X, op=AL.add)

        # ================= bulk phase: per-sample cost -> EC =================
        for b in range(BL):
            pn = spool.tile([128, RT, F], f32, tag="pn")
            tn = spool.tile([128, RT, F], f32, tag="tn")
            nc.sync.dma_start(pn[:], pred_d[b].rearrange("(a p) f -> p a f", p=128))
            nc.sync.dma_start(tn[:], targ_d[b].rearrange("(a p) f -> p a f", p=128))

            # transposed cost: d2T[c, r] = y2[c] + x2[r] - 2*(target @ pred^T)
            ttr = spool.tile([128, RT, 128], f32, tag="ttr")   # -2 * target^T
            ptr = spool.tile([128, RT, 128], f32, tag="ptr")   # pred^T
            for rt in range(RT):
                ps = ppool_t.tile([128, 128], f32, tag="pst")
                nc.tensor.matmul(ps[:], tn[:, rt], ident[:],
                                 start=True, stop=True, is_transpose=True)
                nc.scalar.activation(ttr[:, rt], ps[:], AF.Copy, scale=-2.0)
                ps2 = ppool_t.tile([128, 128], f32, tag="pst")
                nc.tensor.matmul(ps2[:], pn[:, rt], ident[:],
                                 start=True, stop=True, is_transpose=True)
                nc.scalar.copy(ptr[:, rt], ps2[:])

            # y2[p, ct] = sum_f target^2 (per target row = d2T partition)
            y2 = spool.tile([128, RT], f32, tag="y2")
            dump = spool.tile([128, F], f32, tag="dump")
            for ct in range(RT):
                nc.vector.scalar_tensor_tensor(
                    dump[:], tn[:, ct], 1.0, tn[:, ct],
                    op0=AL.mult, op1=AL.mult, accum_out=y2[:, ct : ct + 1])
            # x2 flat [1, 512] via ones-matmul over (pred^T)^2
            tsq = spool.tile([128, RT * 128], f32, tag="tsq")
            ptr_flat = ptr[:].rearrange("p a f -> p (a f)")
            nc.vector.tensor_mul(tsq[:], ptr_flat, ptr_flat)
            x2p = ppool_s.tile([1, S], f32, tag="x2p")
            nc.tensor.matmul(x2p[:], ones[:], tsq[:], start=True, stop=True)
            x2s = spool.tile([1, S], f32, tag="x2s")
            nc.scalar.copy(x2s[:], x2p[:])
            x2b = spool.tile([128, S], f32, tag="x2b")
            nc.gpsimd.partition_broadcast(x2b[:], x2s[:])

            d2s = spool.tile([128, RT, S], f32, tag="d2s")  # d2T: [c, ct, r]
            for ct in range(RT):
                mm = ppool.tile([128, S], f32, tag="mm")
                nc.tensor.matmul(mm[:], ttr[:, ct], ptr_flat,
                                 start=True, stop=True)
                # d2T = (-2xy + y2) + x2
                nc.vector.scalar_tensor_tensor(
                    d2s[:, ct], mm[:], y2[:, ct : ct + 1], x2b[:],
                    op0=AL.add, op1=AL.add)
                # diag piece: qdiag[p, ct*BL + b] = d2T[p, ct*128 + p]
                nc.vector.scalar_tensor_tensor(
                    dump[:, 0:128], d2s[:, ct, ct * 128 : (ct + 1) * 128], 1.0,
                    ident[:], op0=AL.mult, op1=AL.mult,
                    accum_out=qdiag[:, ct * BL + b : ct * BL + b + 1])

            # scatter d2T -> skewed ec buffer (j-major): dest partition 16b+s,
            # elem offset j*JP + (r + s - 1) + 1 ... column j of chunk s holds
            # rows contiguously (2KB runs). Source: d2T partition c = global
            # col-1 = (s-1)*W + j, free = r (contiguous 512 within ct blocks).
            for s in range(1, NS + 1):
                c0 = (s - 1) * W
                nj = min(W, S - c0)
                # partition range c0..c0+nj-1 may span two 128-partition
                # ct-tiles of d2s; split at the boundary.
                jlo = 0
                while jlo < nj:
                    cg = c0 + jlo                 # global col-1
                    pt = cg // 128                # which partition tile? no:
                    # d2s partitions are target-col within ct? d2T[c, ct, r]:
                    # partition c covers cols ct*128+c ... so col cg lives at
                    # partition cg % 128 in ct-slab cg // 128 of the FREE dim.
                    jhi = min(nj, (pt + 1) * 128 - c0)
                    npj = jhi - jlo
                    src = d2s[cg % 128 : cg % 128 + npj, cg // 128, :]
                    base = ec[16 * b + s : 16 * b + s + 1, 0:1]
                    dst = bass.AP(
                        base.tensor, base.offset + (2 * s + 1 + jlo * JP),
                        [[base.ap[0][0], 1], [JP, npj], [1, S]])
                    eng = nc.sync if (s + jlo) % 2 == 0 else nc.gpsimd
                    eng.dma_start(dst, src)
                    jlo = jhi

        # sqrt then exp over the whole skewed buffer (in place):
        # C = sqrt(d2); EC = exp(-C). Pad BIG -> sqrt=1e15 -> exp -> 0.
        nc.scalar.activation(ec[:], ec[:], AF.Sqrt)
        # trace: qdiag currently holds diag d2 -> sqrt, then ones-matmul
        nc.scalar.activation(qdiag[:], qdiag[:], AF.Sqrt)
        trp = ppool_s.tile([1, RT * BL], f32, tag="trp")
        nc.tensor.matmul(trp[:], ones[:], qdiag[:], start=True, stop=True)
        trs = pool.tile([1, RT * BL], f32, tag="trs")
        nc.scalar.copy(trs[:], trp[:])
        # sum the RT pieces per sample: cols rt*BL + b -> [1, BL, RT] reduce X
        nc.vector.tensor_reduce(
            trc[:], trs[:].rearrange("o (a b) -> o b a", a=RT),
            axis=mybir.AxisListType.X, op=AL.add)
        nc.scalar.activation(ec[:], ec[:], AF.Exp, scale=-1.0)

        # per-scale factor = exp(4*kappa) = exp(rhat/NAPPLY)
        nc.vector.tensor_scalar(g4f[:], trc[:], TR_A / NAPPLY, TR_B / NAPPLY,
                                op0=AL.mult, op1=AL.add)
        nc.scalar.activation(g4f[:], g4f[:], AF.Exp)
        nc.gpsimd.partition_broadcast(g4bc[:], g4f[:])
        gsel = spool.tile([128, BL], f32, tag="gsel")
        nc.vector.tensor_mul(gsel[:], g4bc[:], bmask[:])
        nc.vector.tensor_reduce(g4[:], gsel[:], axis=mybir.AxisListType.X,
                                op=AL.add)
        nc.vector.tensor_scalar(rhat_t[:], trc[:], TR_A, TR_B,
                                op0=AL.mult, op1=AL.add)

        # ================= serial wavefront (2 rows per step) =================
        # slot layout (72 cols): c0@0, d0@1..35, c1@36, d1@37..71.
        # c_q = left-chunk end value for this step's row q (shuffled in).
        # v_0 pair-sums prev slot's d1 (with prev c1 as the j=0 left value);
        # v_1 pair-sums this step's scan0 output d0 (with c0 at j=0).
        shuf_mask = [(i if i % 16 == 0 else i - 1) for i in range(32)]
        CW = W + 1  # block width
        for t in range(1, NSTEP + 1):
            cur, prev = t % 3, (t - 1) % 3
            # carries: cur c0 <- prev d0 last [p-1]; cur c1 <- prev d1 last
            nc.vector.stream_shuffle(
                zr[:, cur, 0 : SLOT : CW],
                zr[:, prev, W : SLOT : CW], shuf_mask)
            if t % 2 == 0:
                nc.vector.tensor_scalar(zr[:, cur, 0 : SLOT : CW],
                                        zr[:, cur, 0 : SLOT : CW],
                                        g4[:], CLAMP, op0=AL.mult, op1=AL.min)
            # v_0 = prev.d1[j] + prev.(c1|d1)[j-1]
            nc.vector.tensor_add(vt[:], zr[:, prev, CW + 1 : SLOT],
                                 zr[:, prev, CW : SLOT - 1])
            if t % 2 == 0:
                nc.vector.tensor_scalar(vt[:], vt[:], g4[:], CLAMP,
                                        op0=AL.mult, op1=AL.min)
            nc.vector.tensor_tensor_scan(
                zr[:, cur, 1 : CW], vt[:],
                bass.AP(ec[:].tensor, ec[:].offset + 2 * t + 1,
                        [[ec[:].ap[0][0], 128], [JP, W]]),
                zr[:, cur, 0:1], op0=AL.add, op1=AL.mult)
            # v_1 = cur.d0[j] + cur.(c0|d0)[j-1]
            nc.vector.tensor_add(vt[:], zr[:, cur, 1 : CW],
                                 zr[:, cur, 0 : CW - 1])
            nc.vector.tensor_tensor_scan(
                zr[:, cur, CW + 1 : SLOT], vt[:],
                bass.AP(ec[:].tensor, ec[:].offset + 2 * t + 2,
                        [[ec[:].ap[0][0], 128], [JP, W]]),
                zr[:, cur, CW : CW + 1], op0=AL.add, op1=AL.mult)

        # ================= finalize =================
        # answer: z at partition 16b+15, data col (511 - 14*35) = 21 -> slot col 22
        fcol = S - 1 - (NS - 1) * W  # 21
        fs = NSTEP % 3
        zp = ppool_s.tile([1, BL], f32, tag="zp")
        nc.tensor.matmul(zp[:], zr[:, fs, W + 2 + fcol : W + 3 + fcol], selm[:],
                         start=True, stop=True)
        nc.vector.tensor_copy(zfin[:], zp[:])
        nc.sync.dma_start(zf_d[:, :], zfin[:])
        nc.sync.dma_start(rhat_d[:, :], rhat_t[:])

    nc.compile()
    return nc


_NC_CACHE = {}


def _get_nc(debug_outputs=False):
    key = bool(debug_outputs)
    if key not in _NC_CACHE:
        _NC_CACHE[key] = build_core_program(debug_outputs=key)
    return _NC_CACHE[key]


def kernel(pred, target, _debug=False):
    pred = np.asarray(pred, dtype=np.float32)
    target = np.asarray(target, dtype=np.float32)
    nc = _get_nc(_debug)
    in_maps = []
    for c in range(NCORES):
        sl = slice(c * BL, (c + 1) * BL)
        in_maps.append({"pred": np.ascontiguousarray(pred[sl]),
                        "target": np.ascontiguousarray(target[sl])})
    res = run_bass_kernel_spmd(nc, in_maps, list(range(NCORES)))
    zf = np.concatenate([res.results[c]["zf"][0] for c in range(NCORES)])
    rhat = np.concatenate([res.results[c]["rhat"][0] for c in range(NCORES)])
    losses = (rhat.astype(np.float64) - np.log(zf.astype(np.float64))) / 1024.0
    if _debug:
        return np.float32(losses.mean()), {"z": zf, "rhat": rhat, "losses": losses}
    return np.float32(losses.mean())


if __name__ == "__main__":
    rng = np.random.default_rng(0)
    p = rng.standard_normal((B, S, F)).astype(np.float32)
    t = rng.standard_normal((B, S, F)).astype(np.float32)
    out, dbg = kernel(p, t, _debug=True)
    print("loss:", out)
    print("z:", dbg["z"][:8])
    print("rhat:", dbg["rhat"][:8])
    print("losses:", dbg["losses"][:8])
